# revision 11
# baseline (speedup 1.0000x reference)
"""Trainium2 Bass kernel for nn_Loss_39341900431615 (v4).

Reference semantics (B,C,H,W = 16,128,128,128; only tensor[0] is read):
    idx = argmax(tensor[0,0].reshape(-1))        # row-major first max
    x0, y0 = idx // W, idx % W
    wgt[j,k] = (x0-j)^2 + (y0-k)^2               # [H,W]
    out[w] = sum_{j,k} wgt[j,k] * tensor[0,j,k,w]  # [W]

Sharding: j split across 8 cores (16 j-planes each). Each core computes
the argmax redundantly from a replicated f32 map and emits
[R0;R1;R2;R3] (the four fixed-basis partial sums, [4,128]) plus the
on-device argmax flat index; the host does the tiny q-combine
(out = (x0^2+y0^2)R0 - 2x0 R1 - 2y0 R2 + R3 in float64) and sums the
8 per-core partials — the same flavor of epilogue as the partial-sum
it already does.

Structure learned from the v1-v3 traces (19.5/18.9/17.25us measured):
  - exec_time = first body instr .. end of a FIXED 55-round walrus exit
    semaphore sweep (7.28us) + exit barriers (~0.75us). Only the span
    to the out-DMA completion is compressible.
  - DMA is packet-RATE bound at small descriptors: 8KB pkts ~303GB/s,
    2KB ~196, 1KB ~130. bf16 rows cap tslice descriptors at 4KB, so
    klo-chunking (2KB descriptors) loses more stream time than the
    matmul overlap it buys: ship tslice as ONE [128 x 4KB] DMA.
  - The critical chain is ts-stream -> 16 matmuls -> PSUM copy -> out
    DMA -> exit. The argmax path (map -> reduce/STT -> PE transposes
    -> gmax/flat) runs entirely in the shadow: the map goes FIRST on
    the same queue (lands during ts stream), the two PE transposes
    execute in the pm2..ts-land window before the matmuls, and flat is
    ready ~1.5us before the PSUM copy.
  - bf16 moving+stationary matmuls: same 1 cycle/row as f32r, half the
    bytes; whole-pipeline rel err ~4e-3 vs the 2e-2 gate.
  - Only SP/ACT/Pool engines may post DMAs (~0.7us each of
    posting-engine time): scalar posts map then ts, gpsimd posts cmat
    then builds the iota/identity constants, sync posts the out.
  - PSUM tiles are padded to a full 2KB bank so no two share an
    accumulation-group zero region; the DVE reads PSUM only at
    partition base 0 and only one PSUM operand per instruction
    (psB bounces through SBUF).
"""

import sys

for _p in ("/opt/trn_rl_repo", "/opt/pypackages"):
    if _p not in sys.path:
        sys.path.insert(0, _p)

import numpy as np
import ml_dtypes

import concourse.bass as bass
from concourse import bacc
import concourse.tile as tile
from concourse import mybir
from concourse.bass_utils import run_bass_kernel_spmd

B, C, H, W = 16, 128, 128, 128
NCORES = 8
JPER = C // NCORES      # 16 j-planes per core
KLO = 16                # contraction steps per partition
KHI = 8                 # k blocks per partition dim
MPART = 64              # map partitions
MFREE = (H * W) // MPART  # 256 map elems per partition

F32 = mybir.dt.float32
BF16 = mybir.dt.bfloat16
AX = mybir.AxisListType
OP = mybir.AluOpType

_CACHE = {}


def _build_bass():
    nc = bacc.Bacc("TRN2", target_bir_lowering=False, debug=False,
                   num_devices=NCORES, enable_partition_id=False)

    map_d = nc.dram_tensor("map", [MPART, MFREE], F32, kind="ExternalInput")
    cm_d = nc.dram_tensor("cmat", [128, KLO * 4], BF16, kind="ExternalInput")
    ts_d = nc.dram_tensor("ts", [128, KLO * W], BF16, kind="ExternalInput")
    outd = nc.dram_tensor("out", [4, W + 1], F32, kind="ExternalOutput")

    with tile.TileContext(nc) as tc:
        with (
            tc.tile_pool(name="main", bufs=1) as pool,
            tc.tile_pool(name="psum", bufs=1, space="PSUM") as psum_pool,
        ):
            mp = pool.tile([MPART, MFREE], F32)
            cw = pool.tile([128, KLO, 4], BF16)
            st = pool.tile([128, KLO, W], BF16)

            # --- input DMA posts: map first (it gates the argmax and
            # is small), then the single 4KB-descriptor ts stream.
            nc.scalar.dma_start(out=mp[:, :], in_=map_d[:, :])
            nc.scalar.dma_start(
                out=st[:, :, :],
                in_=ts_d.ap().rearrange("p (a b) -> p a b", a=KLO))
            nc.gpsimd.dma_start(
                out=cw[:, :, :],
                in_=cm_d.ap().rearrange("p (a b) -> p a b", a=KLO))

            # --- gpsimd consts in the DMA shadow ---
            flati = pool.tile([MPART, MFREE], F32)
            nc.gpsimd.iota(flati[:, :], [[1, MFREE]], channel_multiplier=MFREE,
                           allow_small_or_imprecise_dtypes=True)
            colr = pool.tile([MPART, MPART], F32)
            nc.gpsimd.iota(colr[:, :], [[1, MPART]], channel_multiplier=0,
                           allow_small_or_imprecise_dtypes=True)
            pid = pool.tile([MPART, 1], F32)
            nc.gpsimd.iota(pid[:, :], [[1, 1]], channel_multiplier=1,
                           allow_small_or_imprecise_dtypes=True)
            r4p = pool.tile([4, W + 1], F32)
            nc.gpsimd.memset(r4p[:, :], 0.0)

            # --- DVE argmax front (gated only by the map DMA); the
            # identity build slots between the reduce and the STT.
            pm2 = pool.tile([MPART, 2], F32)
            nc.vector.tensor_reduce(pm2[:, 0:1], mp[:, :], axis=AX.X,
                                    op=OP.max)
            ident = pool.tile([MPART, MPART], F32)
            nc.vector.tensor_scalar(ident[:, :], colr[:, :], pid[:, 0:1],
                                    None, op0=OP.is_equal)
            dum = pool.tile([MPART, MFREE], F32)
            nc.vector.scalar_tensor_tensor(
                dum, in0=mp[:, :], scalar=pm2[:, 0:1], in1=flati,
                op0=OP.is_equal, op1=OP.mult, accum_out=pm2[:, 1:2])

            # --- PE stream: the two argmax transposes run in the
            # pm2-ready .. ts-landed window, then the 16 matmuls.
            # PSUM tiles padded to one 2KB zero region each.
            psrT = psum_pool.tile([4, 512], F32)
            psr = psrT[:, 0:W]
            psAT = psum_pool.tile([1, 512], F32)
            psA = psAT[:, 0:MPART]
            psBT = psum_pool.tile([1, 512], F32)
            psB = psBT[:, 0:MPART]
            nc.tensor.matmul(psA, pm2[:, 0:1], ident[:, :],
                             is_transpose=True)
            nc.tensor.matmul(psB, pm2[:, 1:2], ident[:, :],
                             is_transpose=True)
            for klo in range(KLO):
                nc.tensor.matmul(psr, cw[:, klo, :], st[:, klo, :],
                                 start=(klo == 0), stop=(klo == KLO - 1))

            # --- DVE: gmax/flat selection off base-0 PSUM rows; flat
            # rides out in column W of the result tile; then the PSUM
            # result copy (single PSUM operand each).
            gmax = pool.tile([1, 1], F32)
            nc.vector.tensor_reduce(gmax, psA, axis=AX.X, op=OP.max)
            sbB = pool.tile([1, MPART], F32)
            nc.vector.tensor_copy(sbB, psB)
            dum2 = pool.tile([1, MPART], F32)
            nc.vector.scalar_tensor_tensor(
                dum2, in0=psA, scalar=gmax[:, 0:1], in1=sbB,
                op0=OP.is_equal, op1=OP.mult, accum_out=r4p[0:1, W:W + 1])
            nc.vector.tensor_copy(r4p[:, 0:W], psr)

            nc.sync.dma_start(out=outd[:, :], in_=r4p[:, :])

    return nc


def _get_bass():
    if "nc" not in _CACHE:
        nc = _build_bass()
        nc.finalize()
        _CACHE["nc"] = nc
    return _CACHE["nc"]


def _host_cmats():
    """Per-core stationary matrices, GLOBAL j coords, bf16."""
    if "cmats" not in _CACHE:
        p = np.arange(128)
        jl = (p // KHI).astype(np.float64)
        kv = ((p % KHI) * KLO)[:, None] + np.arange(KLO)[None, :]
        kv = kv.astype(np.float64)
        mats = []
        for c in range(NCORES):
            jg = jl + c * JPER
            cm = np.empty((128, KLO, 4), dtype=np.float64)
            cm[:, :, 0] = 1.0
            cm[:, :, 1] = jg[:, None]
            cm[:, :, 2] = kv
            cm[:, :, 3] = (jg * jg)[:, None] + kv * kv
            mats.append(np.ascontiguousarray(
                cm.reshape(128, KLO * 4).astype(ml_dtypes.bfloat16)))
        _CACHE["cmats"] = mats
    return _CACHE["cmats"]


def _make_in_maps(tensor):
    t0 = np.ascontiguousarray(tensor[0], dtype=np.float32)  # [C,H,W]
    mp0 = np.ascontiguousarray(t0[0].reshape(MPART, MFREE))
    cmats = _host_cmats()
    in_maps = []
    for c in range(NCORES):
        jlo = c * JPER
        sl = np.ascontiguousarray(
            t0[jlo:jlo + JPER].reshape(128, KLO * W)).astype(ml_dtypes.bfloat16)
        in_maps.append({"map": mp0, "cmat": cmats[c], "ts": sl})
    return in_maps


def _partial_from_out(arr):
    """Decode one core's [4, W+1] result into its [W] partial (f64)."""
    r = np.asarray(arr, dtype=np.float64).reshape(4, W + 1)
    flat = int(round(r[0, W]))
    x0, y0 = flat // W, flat % W
    q = np.array([x0 * x0 + y0 * y0, -2.0 * x0, -2.0 * y0, 1.0])
    return q @ r[:, 0:W]


def kernel(tensor):
    nc = _get_bass()
    res = run_bass_kernel_spmd(nc, _make_in_maps(tensor),
                               core_ids=list(range(NCORES)))
    partials = np.stack([_partial_from_out(r["out"]) for r in res.results])
    return partials.sum(axis=0).astype(np.float32)
